# revision 15
# baseline (speedup 1.0000x reference)
"""Trainium2 Bass kernel: DiscreteEmbedding (rect-window embedding lookup).

Math (matches the jax reference):
    xs  = x * 2048;  y = xs + 0.5
    i_lo = ceil(y)-1  (robust fp32 compare fixup for the HW int round mode)
    plain tokens:    out = T[i_lo]           (T[2048] := 0 for the xs>2047.5 tail)
    boundary tokens: out = 0.5*(T[i_lo] + T[i_lo+1])   (y exactly integer)

Device strategy (8 cores, data-parallel over tokens; 8192 tokens/core):
  - ONE bf16 gather per token from an INTERLEAVED pair table in DRAM:
      tcd[2v]   = bf16(T[v])          v = 0..2047
      tcd[2v+1] = bf16(0.5*(T[v]+T[v+1]))   (T[2048]=0)
      tcd[4096] = 0                   (the xs>2047.5 tail)
    so idx' = 2*i_lo + boundary -- two DVE ops, no floor/mod remap.
  - Block-major SBUF layout (partition p holds rows 16p..16p+15): the avg
    operand T[w+1] is an intra-partition shifted view; only the 16 rows
    crossing partitions need the tiny strided `tnext` load.  Every big DMA
    (x, table load, table store, output stores) is contiguous per
    partition -- no descriptor storms on the rings.
  - Gathers run triggered on 4 SWDGE queues (desc-gen ~8ns/idx/queue is
    the critical path).  Warmup gathers are the first Pool ops so the Q7
    IRAM load (~10us) overlaps the preamble + table build; trailing table
    dependency is only the contiguous 1MB store.
  - ACT computes th=0.5*T while DVE interleaves idx math with the bf16
    cast/avg so both finish inside the IRAM window.
  - Host: un-permute gather positions to token order, upcast bf16->fp32
    (layout/dtype only; every value is HW-produced).
"""

import numpy as np
import ml_dtypes

import concourse.mybir as mybir
import concourse.tile as tile
from concourse.tile import add_dep_helper
from concourse import bacc, bass_utils

N_CORES = 8
B, S = 32, 2048
V, D = 2048, 128
TOK = B * S
TPC = TOK // N_CORES        # 8192 tokens per core
SPC = TPC // 16             # 512
VEXT = 4224                 # pair table rows (4096 pairs + zero row + pad)
NQ = 4
CHUNK = 2048
NCHUNK = TPC // CHUNK       # 4: one chunk per queue, no 2nd-round SEQ block
JB = CHUNK // 128           # 16 j-blocks per chunk

F32 = mybir.dt.float32
I32 = mybir.dt.int32
I16 = mybir.dt.int16
BF16 = mybir.dt.bfloat16
OP = mybir.AluOpType
AF = mybir.ActivationFunctionType


def build():
    nc = bacc.Bacc(
        "TRN2",
        target_bir_lowering=False,
        debug=False,
        num_devices=N_CORES,
        num_swdge_queues=NQ,
    )
    xr = nc.dram_tensor("xr", [128, SPC], F32, kind="ExternalInput")
    emb = nc.dram_tensor("emb", [V, D], F32, kind="ExternalInput")
    out = nc.dram_tensor("out", [128, (TPC // 128) * D], BF16, kind="ExternalOutput")
    tcd = nc.dram_tensor("tcd", [VEXT, D], BF16, kind="Internal")

    with tile.TileContext(nc) as tc:
        with tc.tile_pool(name="sb", bufs=1) as sb, tc.tile_pool(name="g", bufs=1) as gp:
            # ---- warmups ASAP: Q7 dma_gather IRAM load (~10us) + ring init
            # overlap the preamble/loads.  Keep Pool pre-ops minimal.
            zidx = sb.tile([128, 1], I16)
            nc.vector.memset(zidx[:], 0)
            wreg = nc.gpsimd.to_reg(16)
            for q in range(NQ):
                wg = sb.tile([128, D], F32, tag=f"warm{q}")
                nc.gpsimd.dma_gather(
                    wg[:].rearrange("p (j d) -> p j d", d=D),
                    emb[:],
                    zidx[:, 0:1],
                    num_idxs=16,
                    num_idxs_reg=wreg,
                    elem_size=D,
                    single_packet=False,
                    queue_num=q,
                )

            # ---- loads: x + table halves split across both rings ----
            xt = sb.tile([128, SPC], F32)
            nc.sync.dma_start(out=xt[0:64, :], in_=xr[0:64, :])
            nc.scalar.dma_start(out=xt[64:128, :], in_=xr[64:128, :])
            tbl32 = sb.tile([128, 16 * D], F32)   # row 16p+n at (p, n)
            nc.sync.dma_start(
                out=tbl32[0:64, :],
                in_=emb[0 : V // 2].rearrange("(p n) d -> p (n d)", p=64),
            )
            nc.scalar.dma_start(
                out=tbl32[64:128, :],
                in_=emb[V // 2 : V].rearrange("(p n) d -> p (n d)", p=64),
            )
            tnext = sb.tile([128, D], F32)        # T[16(p+1)], T[2048]=0
            nc.vector.memset(tnext[:], 0.0)
            nc.sync.dma_start(
                out=tnext[0:127, :],
                in_=emb[16:V].rearrange("(p n) d -> p (n d)", p=127)[:, 0:D],
            )
            zrow = sb.tile([1, D], BF16)
            nc.vector.memset(zrow[:], 0.0)

            # ---- DVE: idx compares first (x lands before the table) ----
            y = sb.tile([128, SPC], F32)
            nc.vector.tensor_scalar(y[:], xt[:], 2048.0, 0.5, op0=OP.mult, op1=OP.add)
            i0 = sb.tile([128, SPC], I32)
            nc.vector.tensor_copy(i0[:], y[:])
            f0 = sb.tile([128, SPC], F32)
            nc.vector.tensor_copy(f0[:], i0[:])
            lt = sb.tile([128, SPC], F32)
            nc.vector.tensor_tensor(lt[:], f0[:], y[:], op=OP.is_lt)
            bnd = sb.tile([128, SPC], F32)
            nc.vector.tensor_tensor(bnd[:], f0[:], y[:], op=OP.is_equal)

            # ---- table ops (DVE) as soon as tbl32 lands ----
            # interleaved pair table in SBUF: slot 2n = T[16p+n], 2n+1 = avg
            tcbf = sb.tile([128, 32 * D], BF16)
            t4 = tcbf[:].rearrange("p (n t d) -> p n t d", t=2, d=D)
            nc.vector.tensor_copy(
                t4[:, :, 0:1, :], tbl32[:].rearrange("p (n u d) -> p n u d", u=1, d=D)
            )
            th = sb.tile([128, 16 * D], F32)      # 0.5*T, the avg operand
            nc.vector.tensor_scalar_mul(th[:], tbl32[:], 0.5)
            thn = sb.tile([128, D], F32)
            nc.vector.tensor_scalar_mul(thn[:], tnext[:], 0.5)
            nc.vector.tensor_tensor(
                t4[:, 0:15, 1:2, :],
                th[:, 0 : 15 * D].rearrange("p (n u d) -> p n u d", u=1, d=D),
                th[:, D : 16 * D].rearrange("p (n u d) -> p n u d", u=1, d=D),
                op=OP.add,
            )
            nc.vector.tensor_tensor(
                t4[:, 15:16, 1:2, :],
                th[:, 15 * D : 16 * D].rearrange("p (n u d) -> p n u d", u=1, d=D),
                thn[:].rearrange("p (n u d) -> p n u d", u=1, d=D),
                op=OP.add,
            )

            # ---- table stores (contiguous per partition, both rings) ----
            st_ta = nc.sync.dma_start(
                out=tcd[0:V].rearrange("(p w) d -> p (w d)", p=64),
                in_=tcbf[0:64, :],
            )
            st_tb = nc.scalar.dma_start(
                out=tcd[V : 2 * V].rearrange("(p w) d -> p (w d)", p=64),
                in_=tcbf[64:128, :],
            )
            st_z = nc.sync.dma_start(out=tcd[2 * V : 2 * V + 1, :], in_=zrow[:])
            tc_stores = [st_ta, st_tb, st_z]

            # rest of idx chain: idx' = 2*(f0 + lt - 1) + bnd
            lf = sb.tile([128, SPC], F32)
            nc.vector.scalar_tensor_tensor(
                out=lf[:], in0=lt[:], scalar=-1.0, in1=f0[:], op0=OP.add, op1=OP.add
            )
            idxf = sb.tile([128, SPC], F32)
            nc.vector.scalar_tensor_tensor(
                out=idxf[:], in0=lf[:], scalar=2.0, in1=bnd[:], op0=OP.mult, op1=OP.add
            )
            idx16 = sb.tile([128, SPC], I16)
            nc.vector.tensor_copy(idx16[:], idxf[:])

            # ---- chunked gathers (triggered), round-robin over queues ----
            nreg = nc.gpsimd.to_reg(CHUNK)
            for ci in range(NCHUNK):
                q = ci % NQ
                g = gp.tile([128, JB * D], BF16, tag=f"g{ci}")
                gi = nc.gpsimd.dma_gather(
                    g[:].rearrange("p (j d) -> p j d", d=D),
                    tcd[0 : 2 * V + 1],
                    idx16[:, ci * (CHUNK // 16) : (ci + 1) * (CHUNK // 16)],
                    num_idxs=CHUNK,
                    num_idxs_reg=nreg,
                    elem_size=D,
                    single_packet=False,
                    queue_num=q,
                )
                if ci == 0:
                    # RAW guard: gather SDMA reads tcd; Tile does not thread
                    # DRAM deps.  Pool program order covers later chunks.
                    for st in tc_stores:
                        add_dep_helper(gi.ins, st.ins, True, "tcd RAW guard")
                eng = nc.sync if ci % 2 == 0 else nc.scalar
                eng.dma_start(
                    out=out[:, ci * JB * D : (ci + 1) * JB * D], in_=g[:]
                )
    nc.compile()
    return nc


_NC = None


def _pos_tok():
    """token handled by gather position i: (i%16)*512 + i//16."""
    i = np.arange(TPC)
    return (i % 16) * SPC + i // 16


def kernel(x, time_embedding):
    global _NC
    x = np.ascontiguousarray(np.asarray(x, dtype=np.float32))
    t = np.ascontiguousarray(np.asarray(time_embedding, dtype=np.float32))
    xf = x.reshape(-1)
    in_maps = []
    for c in range(N_CORES):
        xc = xf[c * TPC : (c + 1) * TPC].reshape(16, SPC)
        in_maps.append({"xr": np.ascontiguousarray(np.tile(xc, (8, 1))), "emb": t})

    if _NC is None:
        _NC = build()
    res = bass_utils.run_bass_kernel_spmd(_NC, in_maps, core_ids=list(range(N_CORES)))
    global _LAST_RES
    _LAST_RES = res

    tok = _pos_tok()  # position i -> token
    outs = []
    for c in range(N_CORES):
        oc = np.asarray(res.results[c]["out"]).astype(np.float32)
        # oc[p, j*D:(j+1)*D] = gather position i = j*128 + p
        pos = oc.reshape(128, TPC // 128, D).transpose(1, 0, 2).reshape(TPC, D)
        full = np.empty_like(pos)
        full[tok] = pos
        outs.append(full)
    return np.concatenate(outs, axis=0).reshape(B, S, D)


# revision 17
# speedup vs baseline: 1.1205x; 1.1205x over previous
"""Trainium2 Bass kernel: DiscreteEmbedding (rect-window embedding lookup).

Math (matches the jax reference):
    xs  = x * 2048;  y = xs + 0.5
    i_lo = ceil(y)-1  (robust fp32 compare fixup for the HW int round mode)
    plain tokens:    out = T[i_lo]           (T[2048] := 0 for the xs>2047.5 tail)
    boundary tokens: out = 0.5*(T[i_lo] + T[i_lo+1])   (y exactly integer)

Device strategy (8 cores, data-parallel over tokens; 8192 tokens/core):
  - ONE bf16 gather per token from an INTERLEAVED pair table in DRAM:
      tcd[2v]   = bf16(T[v])          v = 0..2047
      tcd[2v+1] = bf16(0.5*(T[v]+T[v+1]))   (T[2048]=0)
      tcd[4096] = 0                   (the xs>2047.5 tail)
    so idx' = 2*i_lo + boundary -- two DVE ops, no floor/mod remap.
  - Block-major SBUF layout (partition p holds rows 16p..16p+15): the avg
    operand T[w+1] is an intra-partition shifted view; only the 16 rows
    crossing partitions need the tiny strided `tnext` load.  Every big DMA
    (x, table load, table store, output stores) is contiguous per
    partition -- no descriptor storms on the rings.
  - Gathers run triggered on 4 SWDGE queues (desc-gen ~8ns/idx/queue is
    the critical path).  Warmup gathers are the first Pool ops so the Q7
    IRAM load (~10us) overlaps the preamble + table build; trailing table
    dependency is only the contiguous 1MB store.
  - ACT computes th=0.5*T while DVE interleaves idx math with the bf16
    cast/avg so both finish inside the IRAM window.
  - Host: un-permute gather positions to token order, upcast bf16->fp32
    (layout/dtype only; every value is HW-produced).
"""

import numpy as np
import ml_dtypes

import concourse.mybir as mybir
import concourse.tile as tile
from concourse.tile import add_dep_helper
from concourse import bacc, bass_utils

N_CORES = 8
B, S = 32, 2048
V, D = 2048, 128
TOK = B * S
TPC = TOK // N_CORES        # 8192 tokens per core
SPC = TPC // 16             # 512
VEXT = 4224                 # pair table rows (4096 pairs + zero row + pad)
NQ = 4
CHUNK = 2048
NCHUNK = TPC // CHUNK       # 4: one chunk per queue, no 2nd-round SEQ block
JB = CHUNK // 128           # 16 j-blocks per chunk

F32 = mybir.dt.float32
I32 = mybir.dt.int32
I16 = mybir.dt.int16
BF16 = mybir.dt.bfloat16
OP = mybir.AluOpType
AF = mybir.ActivationFunctionType


def build():
    nc = bacc.Bacc(
        "TRN2",
        target_bir_lowering=False,
        debug=False,
        num_devices=N_CORES,
        num_swdge_queues=NQ,
    )
    xr = nc.dram_tensor("xr", [128, SPC], F32, kind="ExternalInput")
    emb = nc.dram_tensor("emb", [V, D], F32, kind="ExternalInput")
    out = nc.dram_tensor("out", [128, (TPC // 128) * D], BF16, kind="ExternalOutput")
    tcd = nc.dram_tensor("tcd", [VEXT, D], BF16, kind="Internal")

    with tile.TileContext(nc) as tc:
        with tc.tile_pool(name="sb", bufs=1) as sb, tc.tile_pool(name="g", bufs=1) as gp:
            # ---- warmups ASAP: Q7 dma_gather IRAM load (~10us) + ring init
            # overlap the preamble/loads.  Keep Pool pre-ops minimal.
            zidx = sb.tile([128, 1], I16)
            nc.vector.memset(zidx[:], 0)
            wreg = nc.gpsimd.to_reg(16)
            for q in range(NQ):
                wg = sb.tile([128, D], F32, tag=f"warm{q}")
                nc.gpsimd.dma_gather(
                    wg[:].rearrange("p (j d) -> p j d", d=D),
                    emb[:],
                    zidx[:, 0:1],
                    num_idxs=16,
                    num_idxs_reg=wreg,
                    elem_size=D,
                    single_packet=False,
                    queue_num=q,
                )

            # ---- loads: x whole on SP (one sem for the idx chain), table
            # halves split across both rings ----
            xt = sb.tile([128, SPC], F32)
            nc.sync.dma_start(out=xt[:], in_=xr[:])
            tbl32 = sb.tile([128, 16 * D], F32)   # row 16p+n at (p, n)
            nc.scalar.dma_start(
                out=tbl32[64:128, :],
                in_=emb[V // 2 : V].rearrange("(p n) d -> p (n d)", p=64),
            )
            nc.sync.dma_start(
                out=tbl32[0:64, :],
                in_=emb[0 : V // 2].rearrange("(p n) d -> p (n d)", p=64),
            )
            tnext = sb.tile([128, D], F32)        # T[16(p+1)], T[2048]=0
            nc.vector.memset(tnext[:], 0.0)
            nc.sync.dma_start(
                out=tnext[0:127, :],
                in_=emb[16:V].rearrange("(p n) d -> p (n d)", p=127)[:, 0:D],
            )
            zrow = sb.tile([1, D], BF16)
            nc.vector.memset(zrow[:], 0.0)

            # ---- DVE: idx compares first (x lands before the table) ----
            y = sb.tile([128, SPC], F32)
            nc.vector.tensor_scalar(y[:], xt[:], 2048.0, 0.5, op0=OP.mult, op1=OP.add)
            i0 = sb.tile([128, SPC], I32)
            nc.vector.tensor_copy(i0[:], y[:])
            f0 = sb.tile([128, SPC], F32)
            nc.vector.tensor_copy(f0[:], i0[:])
            lt = sb.tile([128, SPC], F32)
            nc.vector.tensor_tensor(lt[:], f0[:], y[:], op=OP.is_lt)
            bnd = sb.tile([128, SPC], F32)
            nc.vector.tensor_tensor(bnd[:], f0[:], y[:], op=OP.is_equal)

            # interleaved pair table in SBUF: slot 2n = T[16p+n], 2n+1 = avg.
            # Plain-slot bf16 cast on ACT (idle), avg chain on DVE.
            tcbf = sb.tile([128, 32 * D], BF16)
            t4 = tcbf[:].rearrange("p (n t d) -> p n t d", t=2, d=D)
            nc.scalar.activation(
                t4[:, :, 0:1, :],
                tbl32[:].rearrange("p (n u d) -> p n u d", u=1, d=D),
                AF.Copy,
            )
            th = sb.tile([128, 16 * D], F32)      # 0.5*T, the avg operand
            nc.vector.tensor_scalar_mul(th[:], tbl32[:], 0.5)
            thn = sb.tile([128, D], F32)
            nc.vector.tensor_scalar_mul(thn[:], tnext[:], 0.5)
            nc.vector.tensor_tensor(
                t4[:, 0:15, 1:2, :],
                th[:, 0 : 15 * D].rearrange("p (n u d) -> p n u d", u=1, d=D),
                th[:, D : 16 * D].rearrange("p (n u d) -> p n u d", u=1, d=D),
                op=OP.add,
            )
            nc.vector.tensor_tensor(
                t4[:, 15:16, 1:2, :],
                th[:, 15 * D : 16 * D].rearrange("p (n u d) -> p n u d", u=1, d=D),
                thn[:].rearrange("p (n u d) -> p n u d", u=1, d=D),
                op=OP.add,
            )

            # rest of idx chain: idx' = 2*(f0 + lt - 1) + bnd
            lf = sb.tile([128, SPC], F32)
            nc.vector.scalar_tensor_tensor(
                out=lf[:], in0=lt[:], scalar=-1.0, in1=f0[:], op0=OP.add, op1=OP.add
            )
            idxf = sb.tile([128, SPC], F32)
            nc.vector.scalar_tensor_tensor(
                out=idxf[:], in0=lf[:], scalar=2.0, in1=bnd[:], op0=OP.mult, op1=OP.add
            )
            idx16 = sb.tile([128, SPC], I16)
            nc.vector.tensor_copy(idx16[:], idxf[:])

            # ---- table stores (contiguous per partition, both rings) ----
            st_ta = nc.sync.dma_start(
                out=tcd[0:V].rearrange("(p w) d -> p (w d)", p=64),
                in_=tcbf[0:64, :],
            )
            st_tb = nc.scalar.dma_start(
                out=tcd[V : 2 * V].rearrange("(p w) d -> p (w d)", p=64),
                in_=tcbf[64:128, :],
            )
            st_z = nc.sync.dma_start(out=tcd[2 * V : 2 * V + 1, :], in_=zrow[:])
            tc_stores = [st_ta, st_tb, st_z]

            # ---- chunked gathers (triggered), round-robin over queues ----
            nreg = nc.gpsimd.to_reg(CHUNK)
            for ci in range(NCHUNK):
                q = ci % NQ
                g = gp.tile([128, JB * D], BF16, tag=f"g{ci}")
                gi = nc.gpsimd.dma_gather(
                    g[:].rearrange("p (j d) -> p j d", d=D),
                    tcd[0 : 2 * V + 1],
                    idx16[:, ci * (CHUNK // 16) : (ci + 1) * (CHUNK // 16)],
                    num_idxs=CHUNK,
                    num_idxs_reg=nreg,
                    elem_size=D,
                    single_packet=False,
                    queue_num=q,
                )
                # RAW guard on EVERY chunk: gather SDMA reads tcd and Tile
                # neither threads DRAM deps nor preserves Pool program order
                # for unguarded chunks (observed reordering).
                for st in tc_stores:
                    add_dep_helper(gi.ins, st.ins, True, "tcd RAW guard")
                eng = nc.sync if ci % 2 == 0 else nc.scalar
                eng.dma_start(
                    out=out[:, ci * JB * D : (ci + 1) * JB * D], in_=g[:]
                )
    nc.compile()
    return nc


_NC = None


def _pos_tok():
    """token handled by gather position i: (i%16)*512 + i//16."""
    i = np.arange(TPC)
    return (i % 16) * SPC + i // 16


def kernel(x, time_embedding):
    global _NC
    x = np.ascontiguousarray(np.asarray(x, dtype=np.float32))
    t = np.ascontiguousarray(np.asarray(time_embedding, dtype=np.float32))
    xf = x.reshape(-1)
    in_maps = []
    for c in range(N_CORES):
        xc = xf[c * TPC : (c + 1) * TPC].reshape(16, SPC)
        in_maps.append({"xr": np.ascontiguousarray(np.tile(xc, (8, 1))), "emb": t})

    if _NC is None:
        _NC = build()
    res = bass_utils.run_bass_kernel_spmd(_NC, in_maps, core_ids=list(range(N_CORES)))
    global _LAST_RES
    _LAST_RES = res

    tok = _pos_tok()  # position i -> token
    outs = []
    for c in range(N_CORES):
        oc = np.asarray(res.results[c]["out"]).astype(np.float32)
        # oc[p, j*D:(j+1)*D] = gather position i = j*128 + p
        pos = oc.reshape(128, TPC // 128, D).transpose(1, 0, 2).reshape(TPC, D)
        full = np.empty_like(pos)
        full[tok] = pos
        outs.append(full)
    return np.concatenate(outs, axis=0).reshape(B, S, D)
